# revision 12
# baseline (speedup 1.0000x reference)
"""Two-layer GCN (PyG GCNConv semantics) on 8 Trainium2 NeuronCores.

Math: out = Ahat @ relu(Ahat @ (X@W1) + b1) @ W2 + b2, with
Ahat = D^-1/2 (A + I) D^-1/2.  The edge normalization factors as
dinv[src]*dinv[dst], so per layer we:
  - pre-scale the source table rows by dinv (folded into PSUM eviction),
  - aggregate with a 0/1-times-dinv_dst one-hot matrix per 128-edge tile
    (segmented matmul on the PE, edges sorted by dst),
  - gather source rows from HBM by index via gpsimd dma_gather (int16
    indices, so the 50000-row table is addressed as two 25000-row halves).

Sharding: destination nodes are split across the 8 cores (6250 each).
Layer-1 dense matmul X@W1 is replicated on every core (cheaper than an
all-gather of the table).  One AllGather shares the layer-2 source table.
"""

import sys

import numpy as np

try:
    import concourse.bass as bass  # noqa: F401
except ImportError:
    sys.path.insert(0, "/opt/trn_rl_repo")

from contextlib import ExitStack

import ml_dtypes

import concourse.bass as bass
import concourse.tile as tile
from concourse import bacc, mybir
from concourse.bass_utils import run_bass_kernel_spmd

BF16 = ml_dtypes.bfloat16

# debug ablation: 0 = no dma_gather + no collective, 1 = gather + no collective,
# 2 = full kernel
ABLATE = 2
SHARED = 0  # allocate t2full with addr_space="Shared"
SP = False  # single_packet on dma_gather
NSWQ = 4  # SWDGE queues (1-4); gathers round-robin across them
GCH = 4  # tiles (128 idx each) per dma_gather call
SCRATCH = 16384  # dynamic DMA scratch (descriptor ring) bytes; 16 B/idx
SRCSORT = 1  # sort edges by src within each (block, half) group

N = 50000
E = 800000
FIN = 128
HID = 128
FOUT = 64
NCORES = 8
NSH = N // NCORES  # 6250 destination nodes per core
BLK = 128  # dst block (psum window)
NBLK = (NSH + BLK - 1) // BLK  # 49
SBW = 4  # dst blocks per superblock (one 512-wide psum bank)
NSB = (NBLK + SBW - 1) // SBW  # 13
HALF = 25000  # table half split (int16 gather indices)
NPAD = ((N + 127) // 128) * 128  # 50048
NDTILES = NPAD // 128  # 391


def _layout(tiles):
    """Static program layout from per-(block,half) tile counts.

    Returns (TT, tile_base[NBLK][2], seg: {(sb,h): (tile0, ntiles)}).
    Data/program order: for sb, for half, for block in sb, k tiles.
    """
    tile_base = np.zeros((NBLK, 2), dtype=np.int64)
    seg = {}
    pos = 0
    for sb in range(NSB):
        blocks = range(sb * SBW, min((sb + 1) * SBW, NBLK))
        for h in (0, 1):
            seg_start = pos
            for b in blocks:
                tile_base[b][h] = pos
                pos += int(tiles[b][h])
            seg[(sb, h)] = (seg_start, pos - seg_start)
    return int(pos), tile_base, seg


def _prep(edge_index):
    src = np.asarray(edge_index[0], dtype=np.int64)
    dst = np.asarray(edge_index[1], dtype=np.int64)
    deg = (np.bincount(dst, minlength=N) + 1).astype(np.float64)
    dinv = (1.0 / np.sqrt(deg)).astype(np.float32)

    s_all = np.concatenate([src, np.arange(N, dtype=np.int64)])
    d_all = np.concatenate([dst, np.arange(N, dtype=np.int64)])
    core = d_all // NSH
    local = d_all % NSH
    block = local // BLK
    sbk = block // SBW
    half = (s_all >= HALF).astype(np.int64)

    cidx = (core * NBLK + block) * 2 + half
    cnt = np.bincount(cidx, minlength=NCORES * NBLK * 2).reshape(NCORES, NBLK, 2)
    tiles = ((cnt + BLK - 1) // BLK).max(axis=0)  # [NBLK, 2] max over cores

    TT, tile_base, seg = _layout(tiles)
    S = TT * BLK

    # sort edges into (core, sb, half, block) segment order; within a group
    # the slot order is free — ascending src improves HBM gather locality
    order = np.lexsort((s_all if SRCSORT else local, block, half, sbk, core))
    s_s = s_all[order]
    d_s = d_all[order]
    core_s = core[order]
    block_s = block[order]
    half_s = half[order]

    gid = (core_s * NBLK + block_s) * 2 + half_s
    change = np.r_[True, gid[1:] != gid[:-1]]
    gstart = np.maximum.accumulate(np.where(change, np.arange(len(gid)), 0))
    rank = np.arange(len(gid)) - gstart
    slot = tile_base[block_s, half_s] * BLK + rank  # per-core slot in [0, S)

    src_loc = np.where(half_s == 0, s_s, s_s - HALF).astype(np.int16)
    dst_loc = (d_s % NSH - block_s * BLK).astype(np.float32)  # 0..127
    dinv_d = dinv[d_s] * dinv[s_s]  # full edge norm dinv_src*dinv_dst

    seg_slot0 = np.zeros((NSB, 2), dtype=np.int64)
    for (sb, h), (t0, _nt) in seg.items():
        seg_slot0[sb, h] = t0 * BLK

    meta_np = np.zeros((NCORES, 128, TT, 2), dtype=np.float32)
    idx_np = np.zeros((NCORES, 128, S // 16), dtype=np.int16)
    for c in range(NCORES):
        m = core_s == c
        sl = slot[m]
        tt = sl // BLK
        pp = sl % BLK
        meta_np[c, pp, tt, 0] = dst_loc[m]
        meta_np[c, pp, tt, 1] = dinv_d[m]
        seg0 = seg_slot0[block_s[m] // SBW, half_s[m]]
        j = sl - seg0
        col = seg0 // 16 + j // 16
        row = j % 16
        v = src_loc[m]
        for g in range(8):  # replicate across the 8 gpsimd 16-partition groups
            idx_np[c, row + 16 * g, col] = v

    dinv_blk = np.zeros((NCORES, 128, NBLK), dtype=np.float32)
    ids = np.arange(NBLK * 128)
    valid = ids < NSH
    for c in range(NCORES):
        tmp = np.zeros(NBLK * 128, np.float32)
        tmp[valid] = dinv[c * NSH + ids[valid]]
        dinv_blk[c] = tmp.reshape(NBLK, 128).T

    tmp2 = np.zeros(NPAD, np.float32)
    tmp2[:N] = dinv
    dinv_dense = np.ascontiguousarray(tmp2.reshape(NDTILES, 128).T)  # [128, NDTILES]

    return tiles, dinv, meta_np, idx_np, dinv_blk, dinv_dense


def _build(tiles):
    """Build the (single, SPMD) Bacc program for the given tile counts."""
    TT, tile_base, seg = _layout(tiles)
    S = TT * BLK
    f32 = mybir.dt.float32
    bf16 = mybir.dt.bfloat16
    i16 = mybir.dt.int16
    AF = mybir.ActivationFunctionType
    OP = mybir.AluOpType

    nc = bacc.Bacc(
        "TRN2",
        target_bir_lowering=False,
        debug=False,
        num_devices=NCORES,
        num_swdge_queues=NSWQ,
        dynamic_dma_scratch_size=SCRATCH,
    )
    xT = nc.dram_tensor("xT", [128, NPAD], bf16, kind="ExternalInput")
    w1 = nc.dram_tensor("w1", [128, HID], bf16, kind="ExternalInput")
    w2 = nc.dram_tensor("w2", [128, FOUT], bf16, kind="ExternalInput")
    b1r = nc.dram_tensor("b1r", [128, HID], f32, kind="ExternalInput")
    b2r = nc.dram_tensor("b2r", [128, FOUT], f32, kind="ExternalInput")
    iot = nc.dram_tensor("iot", [128, BLK], bf16, kind="ExternalInput")
    meta = nc.dram_tensor("meta", [128, TT, 2], f32, kind="ExternalInput")
    idxt = nc.dram_tensor("idx", [128, S // 16], i16, kind="ExternalInput")
    outp = nc.dram_tensor("out", [NSH, FOUT], f32, kind="ExternalOutput")

    with tile.TileContext(nc) as tc, ExitStack() as ctx:
        const = ctx.enter_context(tc.tile_pool(name="const", bufs=1))
        dram = ctx.enter_context(tc.tile_pool(name="dram", bufs=1, space="DRAM"))
        xin = ctx.enter_context(tc.tile_pool(name="xin", bufs=4))
        t1ev = ctx.enter_context(tc.tile_pool(name="t1ev", bufs=4))
        gpool = ctx.enter_context(tc.tile_pool(name="g", bufs=3))
        tpp = ctx.enter_context(tc.tile_pool(name="tp", bufs=12))
        evp = ctx.enter_context(tc.tile_pool(name="ev", bufs=4))
        psd = ctx.enter_context(tc.tile_pool(name="psd", bufs=2, space="PSUM"))
        psa = ctx.enter_context(tc.tile_pool(name="psa", bufs=4, space="PSUM"))
        pso = ctx.enter_context(tc.tile_pool(name="pso", bufs=2, space="PSUM"))

        def cload(ap, shape, dtype, tag):
            t = const.tile(shape, dtype, tag=tag)
            nc.sync.dma_start(t[:], ap)
            return t

        w1_sb = cload(w1[:, :], [128, HID], bf16, "w1")
        w2_sb = cload(w2[:, :], [128, FOUT], bf16, "w2")
        b1_sb = cload(b1r[:, :], [128, HID], f32, "b1")
        b2_sb = cload(b2r[:, :], [128, FOUT], f32, "b2")
        iota_sb = cload(iot[:, :], [128, BLK], bf16, "iota")
        meta_sb = cload(meta[:, :, :], [128, TT, 2], f32, "meta")
        idx_sb = cload(idxt[:, :], [128, S // 16], i16, "idx")

        table1 = dram.tile([NPAD, HID], bf16, tag="table1")
        t2loc = dram.tile([NSH, HID], bf16, tag="t2loc")
        t2full = dram.tile(
            [N, HID], bf16, tag="t2full", addr_space="Shared" if SHARED else "Local"
        )

        # Phase A: table1 = X @ W1, replicated on every core (norm lives in meta)
        DB = 4  # node tiles per DMA batch
        for j0 in range(0, NDTILES, DB):
            nb = min(DB, NDTILES - j0)
            xt = xin.tile([128, DB * 128], bf16, tag="xt")
            nc.sync.dma_start(
                xt[:, 0 : nb * 128], xT[:, j0 * 128 : (j0 + nb) * 128]
            )
            ev = t1ev.tile([128, DB, HID], bf16, tag="t1ev")
            for i in range(nb):
                ps = psd.tile([128, HID], f32, tag="psd")
                nc.tensor.matmul(
                    ps[:],
                    lhsT=xt[:, i * 128 : (i + 1) * 128],
                    rhs=w1_sb[:],
                    start=True,
                    stop=True,
                )
                nc.scalar.activation(ev[:, i, :], ps[:], AF.Copy)
            # table1 row j*128+p <- ev[p, j-j0, :]
            nc.sync.dma_start(
                table1[j0 * 128 : (j0 + nb) * 128, :].rearrange(
                    "(t p) f -> p t f", p=128
                ),
                ev[:, 0:nb, :],
            )

        qctr = [0]

        def agg(layer):
            table = table1 if layer == 1 else t2full
            for sb in range(NSB):
                blocks = list(range(sb * SBW, min((sb + 1) * SBW, NBLK)))
                nbl = len(blocks)
                gt = {}
                for h in (0, 1):
                    t0, ntl = seg[(sb, h)]
                    if ntl == 0:
                        continue
                    g = gpool.tile([128, ntl, 128], bf16, tag=f"g{h}")
                    if ABLATE >= 1:
                        view = table[0:HALF, :] if h == 0 else table[HALF : 2 * HALF, :]
                        # descriptor ring limit: SCRATCH/16 idx per gather
                        for q0 in range(0, ntl, GCH):
                            qn = min(GCH, ntl - q0)
                            c0 = (t0 + q0) * 8  # idx columns (tile*128/16)
                            nc.gpsimd.dma_gather(
                                out_ap=g[:, q0 : q0 + qn, :],
                                in_ap=view,
                                idxs_ap=idx_sb[:, c0 : c0 + qn * 8],
                                num_idxs=qn * 128,
                                num_idxs_reg=qn * 128,
                                elem_size=HID,
                                queue_num=qctr[0] % NSWQ,
                                single_packet=SP,
                            )
                            qctr[0] += 1
                    else:
                        nc.vector.memset(g[:], 0)
                    gt[h] = g
                for b in blocks:
                    ps = psa.tile([128, BLK], f32, tag="psa")  # one bank per block
                    for h in (0, 1):
                        if seg[(sb, h)][1] == 0 or tiles[b][h] == 0:
                            continue
                        seg_t0 = seg[(sb, h)][0]
                        for k in range(int(tiles[b][h])):
                            t = int(tile_base[b][h]) + k
                            gofs = t - seg_t0
                            first = (k == 0) and (h == 0 or tiles[b][0] == 0)
                            last = (k == int(tiles[b][h]) - 1) and (
                                h == 1 or tiles[b][1] == 0
                            )
                            tp = tpp.tile([128, BLK], bf16, tag="tp")
                            nc.vector.tensor_scalar(
                                out=tp[:],
                                in0=iota_sb[:],
                                scalar1=meta_sb[:, t, 0:1],
                                scalar2=meta_sb[:, t, 1:2],
                                op0=OP.is_equal,
                                op1=OP.mult,
                            )
                            gtile = gt[h][:, gofs, :]
                            if layer == 1:
                                nc.tensor.matmul(
                                    ps[:], lhsT=tp[:], rhs=gtile, start=first, stop=last
                                )
                            else:
                                nc.tensor.matmul(
                                    ps[:], lhsT=gtile, rhs=tp[:], start=first, stop=last
                                )
                    r0 = b * BLK
                    r1 = min(NSH, r0 + BLK)
                    if layer == 1:
                        s1 = evp.tile([128, HID], f32, tag="s1")
                        nc.vector.tensor_add(s1[:], ps[:], b1_sb[:])
                        ev = evp.tile([128, HID], bf16, tag="t2ev")
                        nc.scalar.activation(ev[:], s1[:], AF.Relu)
                        nc.sync.dma_start(t2loc[r0:r1, :], ev[0 : r1 - r0, :])
                    else:
                        ag2 = evp.tile([128, BLK], bf16, tag="ag2")
                        nc.scalar.activation(ag2[:], ps[:], AF.Copy)
                        po = pso.tile([128, FOUT], f32, tag="pso")
                        nc.tensor.matmul(
                            po[:], lhsT=ag2[:], rhs=w2_sb[:], start=True, stop=True
                        )
                        oo = evp.tile([128, FOUT], f32, tag="oo")
                        nc.vector.tensor_add(oo[:], po[:], b2_sb[:])
                        nc.sync.dma_start(outp[r0:r1, :], oo[0 : r1 - r0, :])

        agg(1)
        if ABLATE >= 2:
            nc.gpsimd.collective_compute(
                "AllGather",
                mybir.AluOpType.bypass,
                replica_groups=[list(range(NCORES))],
                ins=[t2loc[:].opt()],
                outs=[t2full[:].opt()],
            )
        else:
            nc.sync.dma_start(t2full[0:NSH, :], t2loc[:, :])
        agg(2)

    nc.finalize()
    return nc


def _in_maps(x, W1, b1, W2, b2, prep):
    tiles, dinv, meta_np, idx_np, dinv_blk, dinv_dense = prep
    xT = np.zeros((128, NPAD), dtype=BF16)
    xT[:, :N] = np.asarray(x, np.float32).T.astype(BF16)
    w1b = np.asarray(W1, np.float32).astype(BF16)
    w2b = np.asarray(W2, np.float32).astype(BF16)
    b1rep = np.broadcast_to(np.asarray(b1, np.float32), (128, HID)).copy()
    b2rep = np.broadcast_to(np.asarray(b2, np.float32), (128, FOUT)).copy()
    iota = np.broadcast_to(np.arange(BLK, dtype=np.float32), (128, BLK)).astype(BF16)
    shared = {
        "xT": xT,
        "w1": w1b,
        "w2": w2b,
        "b1r": b1rep,
        "b2r": b2rep,
        "iot": np.ascontiguousarray(iota),
    }
    return [
        dict(
            shared,
            meta=np.ascontiguousarray(meta_np[c]),
            idx=np.ascontiguousarray(idx_np[c]),
        )
        for c in range(NCORES)
    ]


def kernel(x, edge_index, W1, b1, W2, b2):
    prep = _prep(edge_index)
    nc = _build(prep[0])
    in_maps = _in_maps(x, W1, b1, W2, b2, prep)
    res = run_bass_kernel_spmd(nc, in_maps, core_ids=list(range(NCORES)), trace=False)
    out = np.concatenate(
        [res.results[c]["out"].astype(np.float32) for c in range(NCORES)], axis=0
    )
    return out

